# revision 1
# baseline (speedup 1.0000x reference)
"""Masked attention-weight kernel (dense_transformer) for 8 TRN2 NeuronCores.

Computes, for inputs query/key [32,1024,512] f32, masks [32,1024] i32:
    q = relu(query @ Wq + bq); k = relu(key @ Wk + bk)
    w = softmax((q @ k^T)/sqrt(512) + key_mask_additive) * query_mask
Output: [32, 1024, 1024] f32.

Strategy: pure data-parallel over batch (4 batches/core, no collectives).
Host pre-transposes query/key to [B_local, D, L] and casts to bf16 so every
device matmul is transpose-free; compute is bf16 with f32 PSUM accumulation.
Softmax skips max-subtraction (logits bounded ~+-12; exp is safe in f32).
The key mask is applied additively (-1e6) to the post-relu k-projection,
which makes masked logits ~-2e8 so exp underflows to exactly 0, and the
ACT exp's fused accum_out produces the masked row sum for free.

Per-core pipeline, per batch:
  1. kT[e,j] = relu(Wk.T @ keyT + bk) via PE matmuls -> ACT relu+bias,
     then +mask on DVE/GpSimd (batch 0 runs the matmuls dt-major so the PE
     consumes (wk_dt, xk_dt) DMA pairs in arrival order at cold start).
  2. qT[e,i] likewise.
  3. For each 128-row block: S = qT.T @ kTm (PE, f32 psum), ACT exp with
     fused row-sum, DVE reciprocal * query_mask, DVE scale, DMA out
     (stores alternate between the gpsimd and scalar queues).
"""

import sys

sys.path.insert(0, "/opt/trn_rl_repo")

import numpy as np
import ml_dtypes
from contextlib import ExitStack

import concourse.tile as tile
from concourse import bacc, mybir
from concourse.bass_utils import run_bass_kernel_spmd

P = 128
B, LQ, LK, D = 32, 1024, 1024, 512
NCORES = 8
BL = B // NCORES          # batches per core
NDT = D // P              # contraction tiles for projections
NET = D // P              # output-feature tiles (= S contraction tiles)
NIB = LQ // P             # 128-row blocks of S per batch
NH = LK // 512            # 512-col halves
SCALE = float(1.0 / np.sqrt(D))
MASKC = -1.0e6

F32 = mybir.dt.float32
BF16 = mybir.dt.bfloat16
AF = mybir.ActivationFunctionType

_CACHE = {}


def _body(tc, qT, kT, Wq, Wk, bq, bk, maskb, qm, out):
    nc = tc.nc
    with ExitStack() as ctx:
        consts = ctx.enter_context(tc.tile_pool(name="consts", bufs=1))
        wpool = ctx.enter_context(tc.tile_pool(name="w", bufs=1))
        inpool = ctx.enter_context(tc.tile_pool(name="inp", bufs=2))
        actpool = ctx.enter_context(tc.tile_pool(name="act", bufs=2))
        mpool = ctx.enter_context(tc.tile_pool(name="mask", bufs=2))
        epool = ctx.enter_context(tc.tile_pool(name="exp", bufs=3))
        opool = ctx.enter_context(tc.tile_pool(name="pout", bufs=3))
        stpool = ctx.enter_context(tc.tile_pool(name="stat", bufs=6))
        ppsum = ctx.enter_context(tc.tile_pool(name="ppsum", bufs=2, space="PSUM"))
        spsum = ctx.enter_context(tc.tile_pool(name="spsum", bufs=3, space="PSUM"))

        # Weights on the scalar DMA queue, inputs on sync, small tensors on
        # gpsimd — three queues pull concurrently at cold start.
        wk_sb = [wpool.tile([P, D], BF16, tag=f"wk{dt_}", name=f"wk{dt_}")
                 for dt_ in range(NDT)]
        wq_sb = [wpool.tile([P, D], BF16, tag=f"wq{dt_}", name=f"wq{dt_}")
                 for dt_ in range(NDT)]
        for dt_ in range(NDT):
            nc.scalar.dma_start(
                out=wk_sb[dt_][:], in_=Wk[dt_ * P:(dt_ + 1) * P, :])
        for dt_ in range(NDT):
            nc.scalar.dma_start(
                out=wq_sb[dt_][:], in_=Wq[dt_ * P:(dt_ + 1) * P, :])

        bk_sb = consts.tile([P, NET], F32)
        nc.gpsimd.dma_start(out=bk_sb[:], in_=bk[:])
        bq_sb = consts.tile([P, NET], F32)
        nc.gpsimd.dma_start(out=bq_sb[:], in_=bq[:])

        # PE warmup: 8 dummy matmuls (~3.4us of cold PE busy, exactly one
        # HAM activity window) on scratch tiles while the first input DMAs
        # are in flight, so the clock-gate reaches K=8/8 just before real
        # matmuls start. Results are never read.
        warm_in = consts.tile([P, 512], BF16, name="warm_in")
        nc.vector.memset(warm_in[:], 0.0)
        warm_ps = ppsum.tile([P, 512], F32, tag="proj", name="warm_ps")
        for _ in range(8):
            nc.tensor.matmul(
                warm_ps[:], lhsT=warm_in[:, 0:P], rhs=warm_in[:],
                start=True, stop=True,
            )

        def load_inputs(b):
            xk, xq = [], []
            for dt_ in range(NDT):
                t = inpool.tile([P, LK], BF16, tag=f"xk{dt_}")
                if b == 0 and dt_ == 0:
                    # split so the very first matmul's 128KB dep lands sooner
                    for h in range(NH):
                        nc.sync.dma_start(
                            out=t[:, h * 512:(h + 1) * 512],
                            in_=kT[b, 0:P, h * 512:(h + 1) * 512])
                else:
                    nc.sync.dma_start(
                        out=t[:], in_=kT[b, dt_ * P:(dt_ + 1) * P, :])
                xk.append(t)
            mask_sb = mpool.tile([P, LK], BF16, tag="maskb")
            if b > 0:
                # prefetched with plenty of slack; keep off the sync queue
                nc.gpsimd.dma_start(out=mask_sb[:], in_=maskb[b])
            for dt_ in range(NDT):
                t = inpool.tile([P, LQ], BF16, tag=f"xq{dt_}")
                if b == 0 and dt_ == 0:
                    for h in range(NH):
                        nc.sync.dma_start(
                            out=t[:, h * 512:(h + 1) * 512],
                            in_=qT[b, 0:P, h * 512:(h + 1) * 512])
                else:
                    nc.sync.dma_start(
                        out=t[:], in_=qT[b, dt_ * P:(dt_ + 1) * P, :])
                xq.append(t)
            if b == 0:
                # batch 0: issue after xq so the mask transfer doesn't steal
                # bandwidth from the cold-start critical path (wk/xk pairs)
                nc.sync.dma_start(out=mask_sb[:], in_=maskb[b])
            qm_sb = mpool.tile([P, NIB], F32, tag="qm")
            nc.gpsimd.dma_start(out=qm_sb[:], in_=qm[b])
            return xk, mask_sb, xq, qm_sb

        def relu_epilogue(ps, bias_sb, out_tiles, et, ih, on_dve=False):
            if on_dve:
                # (psum + bias) max 0 — exact relu+bias as one DVE op
                nc.vector.tensor_scalar(
                    out=out_tiles[et][:, ih * 512:(ih + 1) * 512],
                    in0=ps[:],
                    scalar1=bias_sb[:, et:et + 1],
                    scalar2=0.0,
                    op0=mybir.AluOpType.add,
                    op1=mybir.AluOpType.max,
                )
            else:
                nc.scalar.activation(
                    out=out_tiles[et][:, ih * 512:(ih + 1) * 512],
                    in_=ps[:],
                    func=AF.Relu,
                    bias=bias_sb[:, et:et + 1],
                    scale=1.0,
                )

        def proj(xin, w_sb, bias_sb, out_tiles):
            # out_tiles[et] = relu(W[:, et].T @ x + b)
            for et in range(NET):
                for ih in range(NH):
                    ps = ppsum.tile([P, 512], F32, tag="proj")
                    for dt_ in range(NDT):
                        nc.tensor.matmul(
                            ps[:],
                            lhsT=w_sb[dt_][:, et * P:(et + 1) * P],
                            rhs=xin[dt_][:, ih * 512:(ih + 1) * 512],
                            start=(dt_ == 0),
                            stop=(dt_ == NDT - 1),
                        )
                    relu_epilogue(ps, bias_sb, out_tiles, et, ih)

        def proj_coldstart(xin, w_sb, bias_sb, out_tiles, pfx="coldk", epi_ih_major=False, split_epi=False):
            # Batch-0 k-proj only: dt-major order so the PE consumes
            # (wk_dt, xk_dt) DMA pairs in arrival order instead of stalling
            # on wk1-3; all 4 et accumulation groups are open at once,
            # borrowing the (still idle) S-phase psum pool for et 0-2.
            pss = []
            for et in range(NET - 1):
                t = spsum.tile([P, LK], F32, tag="S", name=f"{pfx}ps{et}")
                pss.append([t[:, 0:512], t[:, 512:1024]])
            pss.append([ppsum.tile([P, 512], F32, tag="proj", name=f"{pfx}3a")[:],
                        ppsum.tile([P, 512], F32, tag="proj", name=f"{pfx}3b")[:]])
            for dt_ in range(NDT):
                for et in range(NET):
                    for ih in range(NH):
                        nc.tensor.matmul(
                            pss[et][ih],
                            lhsT=w_sb[dt_][:, et * P:(et + 1) * P],
                            rhs=xin[dt_][:, ih * 512:(ih + 1) * 512],
                            start=(dt_ == 0),
                            stop=(dt_ == NDT - 1),
                        )
            # epi_ih_major: S block 0 needs only the ih=0 halves of qT,
            # so drain those four groups first
            if epi_ih_major:
                order = [(et, ih) for ih in range(NH) for et in range(NET)]
            else:
                order = [(et, ih) for et in range(NET) for ih in range(NH)]
            for n, (et, ih) in enumerate(order):
                relu_epilogue(pss[et][ih], bias_sb, out_tiles, et, ih,
                              on_dve=(split_epi and n % 2 == 1))

        def mask_add(kraw, mask_sb, b):
            kTm = [actpool.tile([P, LK], BF16, tag=f"kTm{et}",
                                name=f"kTm{et}_{b}")
                   for et in range(NET)]
            for et in range(NET):
                # split across gpsimd and vector so neither gates the S phase
                eng = nc.gpsimd if et % 2 == 0 else nc.vector
                eng.tensor_add(kTm[et][:], kraw[et][:], mask_sb[:])
            return kTm

        def s_block(b, ib, qTt, kTm, qm_sb):
            sp = spsum.tile([P, LK], F32, tag="S")
            for et in range(NET):
                for jh in range(NH):
                    nc.tensor.matmul(
                        sp[:, jh * 512:(jh + 1) * 512],
                        lhsT=qTt[et][:, ib * P:(ib + 1) * P],
                        rhs=kTm[et][:, jh * 512:(jh + 1) * 512],
                        start=(et == 0),
                        stop=(et == NET - 1),
                    )
            ex = epool.tile([P, LK], BF16, tag="exp")
            rs = stpool.tile([P, 1], F32, tag="rowsum")
            nc.scalar.activation(
                out=ex[:], in_=sp[:], func=AF.Exp, scale=SCALE,
                accum_out=rs[:],
            )
            rc = stpool.tile([P, 1], F32, tag="recip")
            nc.vector.reciprocal(out=rc[:], in_=rs[:])
            rq = stpool.tile([P, 1], F32, tag="rq")
            nc.vector.tensor_tensor(
                out=rq[:], in0=rc[:], in1=qm_sb[:, ib:ib + 1],
                op=mybir.AluOpType.mult,
            )
            po = opool.tile([P, LK], F32, tag="po")
            nc.vector.tensor_scalar(
                out=po[:], in0=ex[:], scalar1=rq[:], scalar2=None,
                op0=mybir.AluOpType.mult,
            )
            # alternate store queues so the output backlog drains 2x faster
            # (sync, not scalar: scalar's ACT must not stall behind DMA issue)
            eng = nc.gpsimd if ib % 2 == 0 else nc.sync
            eng.dma_start(out=out[b, ib * P:(ib + 1) * P, :], in_=po[:])

        def s_block_final(b, ib, qTt, kTm, qm_sb):
            # Last block of the kernel: jh-major matmuls into two separate
            # 1-bank psums + a fully split epilogue (independent half tiles)
            # so the first half's exp/mul/store overlap the second half's
            # matmuls and exp — shortening the serial tail after the last MM.
            sps = [ppsum.tile([P, 512], F32, tag="proj", name=f"fsp{jh}")
                   for jh in range(NH)]
            rss = [stpool.tile([P, 1], F32, tag=f"rowsum{jh}", name=f"frs{jh}")
                   for jh in range(NH)]
            exs = [epool.tile([P, 512], BF16, tag=f"fex{jh}", name=f"fex{jh}")
                   for jh in range(NH)]
            for jh in range(NH):
                for et in range(NET):
                    nc.tensor.matmul(
                        sps[jh][:],
                        lhsT=qTt[et][:, ib * P:(ib + 1) * P],
                        rhs=kTm[et][:, jh * 512:(jh + 1) * 512],
                        start=(et == 0),
                        stop=(et == NET - 1),
                    )
                nc.scalar.activation(
                    out=exs[jh][:], in_=sps[jh][:],
                    func=AF.Exp, scale=SCALE, accum_out=rss[jh][:],
                )
            rs = stpool.tile([P, 1], F32, tag="rowsumt")
            nc.vector.tensor_tensor(
                out=rs[:], in0=rss[0][:], in1=rss[1][:],
                op=mybir.AluOpType.add)
            rc = stpool.tile([P, 1], F32, tag="recip")
            nc.vector.reciprocal(out=rc[:], in_=rs[:])
            rq = stpool.tile([P, 1], F32, tag="rq")
            nc.vector.tensor_tensor(
                out=rq[:], in0=rc[:], in1=qm_sb[:, ib:ib + 1],
                op=mybir.AluOpType.mult,
            )
            for jh in range(NH):
                poh = opool.tile([P, 512], F32, tag=f"fpo{jh}", name=f"fpo{jh}")
                nc.vector.tensor_scalar(
                    out=poh[:], in0=exs[jh][:],
                    scalar1=rq[:], scalar2=None,
                    op0=mybir.AluOpType.mult,
                )
                eng = nc.gpsimd if jh == 0 else nc.sync
                eng.dma_start(
                    out=out[b, ib * P:(ib + 1) * P, jh * 512:(jh + 1) * 512],
                    in_=poh[:],
                )

        def s_phase(b, qTt, kTm, qm_sb):
            for ib in range(NIB):
                if b == BL - 1 and ib == NIB - 1:
                    s_block_final(b, ib, qTt, kTm, qm_sb)
                else:
                    s_block(b, ib, qTt, kTm, qm_sb)

        cur = load_inputs(0)
        for b in range(BL):
            xk, mask_sb, xq, qm_sb = cur
            kraw = [actpool.tile([P, LK], BF16, tag=f"kraw{et}",
                                 name=f"kraw{et}_{b}")
                    for et in range(NET)]
            if b == 0:
                proj_coldstart(xk, wk_sb, bk_sb, kraw, pfx="coldk")
            else:
                proj(xk, wk_sb, bk_sb, kraw)
            kTm = mask_add(kraw, mask_sb, b)
            qTt = [actpool.tile([P, LQ], BF16, tag=f"qT{et}",
                                name=f"qT{et}_{b}")
                   for et in range(NET)]
            if b == 0:
                proj_coldstart(xq, wq_sb, bq_sb, qTt, pfx="coldq", split_epi=True)
            else:
                proj(xq, wq_sb, bq_sb, qTt)
            if b + 1 < BL:
                cur = load_inputs(b + 1)
            s_phase(b, qTt, kTm, qm_sb)


def _build():
    nc = bacc.Bacc(
        "TRN2",
        target_bir_lowering=False,
        debug=False,
        enable_asserts=False,
        num_devices=NCORES,
    )
    qT = nc.dram_tensor("qT", [BL, D, LQ], BF16, kind="ExternalInput").ap()
    kT = nc.dram_tensor("kT", [BL, D, LK], BF16, kind="ExternalInput").ap()
    Wq = nc.dram_tensor("Wq", [D, D], BF16, kind="ExternalInput").ap()
    Wk = nc.dram_tensor("Wk", [D, D], BF16, kind="ExternalInput").ap()
    bq = nc.dram_tensor("bq", [P, NET], F32, kind="ExternalInput").ap()
    bk = nc.dram_tensor("bk", [P, NET], F32, kind="ExternalInput").ap()
    maskb = nc.dram_tensor("maskb", [BL, P, LK], BF16, kind="ExternalInput").ap()
    qm = nc.dram_tensor("qm", [BL, P, NIB], F32, kind="ExternalInput").ap()
    out = nc.dram_tensor("out", [BL, LQ, LK], F32, kind="ExternalOutput").ap()

    with tile.TileContext(nc) as tc:
        _body(tc, qT, kT, Wq, Wk, bq, bk, maskb, qm, out)
    nc.compile()
    return nc


def _get_nc():
    if "nc" not in _CACHE:
        _CACHE["nc"] = _build()
    return _CACHE["nc"]


def _make_in_maps(query, key, query_mask, key_mask, Wq, bq, Wk, bk):
    bf = ml_dtypes.bfloat16
    query = np.asarray(query, dtype=np.float32)
    key = np.asarray(key, dtype=np.float32)
    query_mask = np.asarray(query_mask)
    key_mask = np.asarray(key_mask)
    Wq_b = np.asarray(Wq, dtype=np.float32).astype(bf)
    Wk_b = np.asarray(Wk, dtype=np.float32).astype(bf)
    # bias for feature e lives at partition e%128, column e//128
    bq_t = np.asarray(bq, dtype=np.float32).reshape(NET, P).T.copy()
    bk_t = np.asarray(bk, dtype=np.float32).reshape(NET, P).T.copy()

    in_maps = []
    for c in range(NCORES):
        sl = slice(c * BL, (c + 1) * BL)
        qTc = query[sl].transpose(0, 2, 1).astype(bf)
        kTc = key[sl].transpose(0, 2, 1).astype(bf)
        mrow = (MASKC * (1 - key_mask[sl])).astype(bf)            # [BL, LK]
        maskb = np.ascontiguousarray(
            np.broadcast_to(mrow[:, None, :], (BL, P, LK))
        )
        qmc = (
            query_mask[sl].astype(np.float32)
            .reshape(BL, NIB, P).transpose(0, 2, 1).copy()
        )
        in_maps.append({
            "qT": qTc, "kT": kTc, "Wq": Wq_b, "Wk": Wk_b,
            "bq": bq_t, "bk": bk_t, "maskb": maskb, "qm": qmc,
        })
    return in_maps


def run(query, key, query_mask, key_mask, Wq, bq, Wk, bk, **kwargs):
    """Run on hardware; returns (output, BassKernelResults)."""
    nc = _get_nc()
    in_maps = _make_in_maps(query, key, query_mask, key_mask, Wq, bq, Wk, bk)
    res = run_bass_kernel_spmd(nc, in_maps, core_ids=list(range(NCORES)), **kwargs)
    outs = [res.results[c]["out"] for c in range(NCORES)]
    full = np.concatenate(outs, axis=0).astype(np.float32, copy=False)
    return full, res


def kernel(query, key, query_mask, key_mask, Wq, bq, Wk, bk):
    full, _ = run(query, key, query_mask, key_mask, Wq, bq, Wk, bk)
    return full



# revision 2
# speedup vs baseline: 1.3904x; 1.3904x over previous
"""Masked attention-weight kernel (dense_transformer) for 8 TRN2 NeuronCores.

Computes, for inputs query/key [32,1024,512] f32, masks [32,1024] i32:
    q = relu(query @ Wq + bq); k = relu(key @ Wk + bk)
    w = softmax((q @ k^T)/sqrt(512) + key_mask_additive) * query_mask
Output: [32, 1024, 1024] f32.

Strategy: data-parallel over batch (4 batches/core, no collectives) PLUS
host-side mask compaction.  Masked key columns have weight exactly 0 in the
reference (exp(-1e9) underflows) and masked query rows are zeroed, so the
host gathers only the valid ~512 query rows / key columns per batch, pads
them to a fixed NQP/NKP (multiple of 128, 640 for this data), and the device
runs dense attention on the compacted [NQP, NKP] problem -- ~2.2x fewer
matmul cycles than the full [1024,1024].  The host scatters the compact
output back into a zero-filled full-size array.

Padded key columns are all-zero inputs, so (with zero bias -- true for this
problem) their projected features are 0, their logits are 0, and each
contributes exp(0)=1 to the softmax row-sum; the device subtracts the
host-provided pad count from the row-sum before taking the reciprocal.
If the key bias were nonzero the host instead ships an additive -1e4
column mask applied to the projected k (use_mask variant).

Per-core pipeline, per batch (all matmuls bf16 with f32 PSUM):
  1. kTm[e,j] = relu(Wk.T @ keyT + bk): PE matmuls in (512,128) psum-bank
     chunks -> relu+bias epilogue alternating ACT/DVE.
  2. qT[e,i] likewise.  Batch 0 runs both projections dt-major across 8
     open psum chains so the PE consumes (w_dt, x_dt) DMA pairs in arrival
     order at cold start.
  3. Per 128-row block: S = qT.T @ kTm (PE), ACT exp with fused row-sum,
     DVE pad-correction + reciprocal, DVE scale, DMA out (stores alternate
     between the gpsimd and sync queues).
"""

import sys

sys.path.insert(0, "/opt/trn_rl_repo")

import numpy as np
import ml_dtypes
from contextlib import ExitStack

import concourse.tile as tile
from concourse import bacc, mybir
from concourse.bass_utils import run_bass_kernel_spmd

P = 128
B, LQ, LK, D = 32, 1024, 1024, 512
NCORES = 8
BL = B // NCORES          # batches per core
NDT = D // P              # contraction tiles for projections
NET = D // P              # output-feature tiles (= S contraction tiles)
SCALE = float(1.0 / np.sqrt(D))
MASKC = -1.0e4

F32 = mybir.dt.float32
BF16 = mybir.dt.bfloat16
AF = mybir.ActivationFunctionType

_CACHE = {}


def _chunks(width):
    """Split a free width into psum-bank-aligned chunks (<=512 each)."""
    out, c0 = [], 0
    while c0 < width:
        cw = min(512, width - c0)
        out.append((c0, cw))
        c0 += cw
    return out


def _body(tc, qT, kT, Wq, Wk, bq, bk, padc, maskc, out, NQP, NKP):
    nc = tc.nc
    NQB = NQP // P            # 128-row S blocks per batch
    SPAD = ((NKP + 511) // 512) * 512   # psum tile width (bank aligned)
    kchunks = _chunks(NKP)
    qchunks = _chunks(NQP)
    use_mask = maskc is not None
    with ExitStack() as ctx:
        consts = ctx.enter_context(tc.tile_pool(name="consts", bufs=1))
        wpool = ctx.enter_context(tc.tile_pool(name="w", bufs=1))
        inpool = ctx.enter_context(tc.tile_pool(name="inp", bufs=2))
        actpool = ctx.enter_context(tc.tile_pool(name="act", bufs=2))
        mpool = ctx.enter_context(tc.tile_pool(name="mask", bufs=2))
        epool = ctx.enter_context(tc.tile_pool(name="exp", bufs=3))
        opool = ctx.enter_context(tc.tile_pool(name="pout", bufs=3))
        stpool = ctx.enter_context(tc.tile_pool(name="stat", bufs=6))
        ppsum = ctx.enter_context(tc.tile_pool(name="ppsum", bufs=2, space="PSUM"))
        spsum = ctx.enter_context(tc.tile_pool(name="spsum", bufs=3, space="PSUM"))

        # Weights on the scalar DMA queue, inputs on sync, small tensors on
        # gpsimd -- three queues pull concurrently at cold start.
        wk_sb = [wpool.tile([P, D], BF16, tag=f"wk{dt_}", name=f"wk{dt_}")
                 for dt_ in range(NDT)]
        wq_sb = [wpool.tile([P, D], BF16, tag=f"wq{dt_}", name=f"wq{dt_}")
                 for dt_ in range(NDT)]
        for dt_ in range(NDT):
            nc.scalar.dma_start(
                out=wk_sb[dt_][:], in_=Wk[dt_ * P:(dt_ + 1) * P, :])
        for dt_ in range(NDT):
            nc.scalar.dma_start(
                out=wq_sb[dt_][:], in_=Wq[dt_ * P:(dt_ + 1) * P, :])

        bk_sb = consts.tile([P, NET], F32)
        nc.gpsimd.dma_start(out=bk_sb[:], in_=bk[:])
        bq_sb = consts.tile([P, NET], F32)
        nc.gpsimd.dma_start(out=bq_sb[:], in_=bq[:])

        # PE warmup: 8 dummy matmuls (~3.4us of cold PE busy, exactly one
        # HAM activity window) on scratch tiles while the first input DMAs
        # are in flight, so the clock-gate reaches K=8/8 just before real
        # matmuls start. Results are never read.
        warm_in = consts.tile([P, 512], BF16, name="warm_in")
        nc.vector.memset(warm_in[:], 0.0)
        warm_ps = ppsum.tile([P, 512], F32, tag="proj", name="warm_ps")
        for _ in range(8):
            nc.tensor.matmul(
                warm_ps[:], lhsT=warm_in[:, 0:P], rhs=warm_in[:],
                start=True, stop=True,
            )

        def load_inputs(b):
            xk, xq = [], []
            for dt_ in range(NDT):
                t = inpool.tile([P, NKP], BF16, tag=f"xk{dt_}")
                if b == 0:
                    # split so chunk-0 matmul deps land sooner at cold start
                    for (c0, cw) in kchunks:
                        nc.sync.dma_start(
                            out=t[:, c0:c0 + cw],
                            in_=kT[b, dt_ * P:(dt_ + 1) * P, c0:c0 + cw])
                else:
                    nc.sync.dma_start(
                        out=t[:], in_=kT[b, dt_ * P:(dt_ + 1) * P, :])
                xk.append(t)
            for dt_ in range(NDT):
                t = inpool.tile([P, NQP], BF16, tag=f"xq{dt_}")
                if b == 0:
                    for (c0, cw) in qchunks:
                        nc.sync.dma_start(
                            out=t[:, c0:c0 + cw],
                            in_=qT[b, dt_ * P:(dt_ + 1) * P, c0:c0 + cw])
                else:
                    nc.sync.dma_start(
                        out=t[:], in_=qT[b, dt_ * P:(dt_ + 1) * P, :])
                xq.append(t)
            pad_sb = mpool.tile([P, 1], F32, tag="padc")
            nc.gpsimd.dma_start(out=pad_sb[:], in_=padc[b])
            mask_sb = None
            if use_mask:
                mask_sb = mpool.tile([P, NKP], BF16, tag="maskc")
                nc.gpsimd.dma_start(out=mask_sb[:], in_=maskc[b])
            return xk, xq, pad_sb, mask_sb

        def relu_epilogue(ps, bias_sb, out_tiles, et, c0, cw, on_dve):
            if on_dve:
                # (psum + bias) max 0 -- exact relu+bias as one DVE op
                nc.vector.tensor_scalar(
                    out=out_tiles[et][:, c0:c0 + cw],
                    in0=ps,
                    scalar1=bias_sb[:, et:et + 1],
                    scalar2=0.0,
                    op0=mybir.AluOpType.add,
                    op1=mybir.AluOpType.max,
                )
            else:
                nc.scalar.activation(
                    out=out_tiles[et][:, c0:c0 + cw],
                    in_=ps,
                    func=AF.Relu,
                    bias=bias_sb[:, et:et + 1],
                    scale=1.0,
                )

        def proj(xin, w_sb, bias_sb, out_tiles, chunks):
            # out_tiles[et] = relu(W[:, et].T @ x + b)
            n = 0
            for et in range(NET):
                for (c0, cw) in chunks:
                    ps = ppsum.tile([P, 512], F32, tag="proj")
                    for dt_ in range(NDT):
                        nc.tensor.matmul(
                            ps[:, 0:cw],
                            lhsT=w_sb[dt_][:, et * P:(et + 1) * P],
                            rhs=xin[dt_][:, c0:c0 + cw],
                            start=(dt_ == 0),
                            stop=(dt_ == NDT - 1),
                        )
                    relu_epilogue(ps[:, 0:cw], bias_sb, out_tiles, et, c0, cw,
                                  on_dve=(n % 2 == 1))
                    n += 1

        def proj_coldstart(xin, w_sb, bias_sb, out_tiles, chunks, pfx):
            # Batch-0 projections: dt-major order so the PE consumes
            # (w_dt, x_dt) DMA pairs in arrival order; all NET*len(chunks)
            # accumulation chains are open at once, borrowing the (still
            # idle) S-phase psum pool.  Chain -> single-bank psum region:
            #   chunk0 (512 wide) x4 et -> spsum tiles 0,1 (two banks each)
            #   chunk1 (<=128)    x4 et -> spsum tile 2 banks + ppsum x2
            sp0 = spsum.tile([P, SPAD], F32, tag="S", name=f"{pfx}c0a")
            sp1 = spsum.tile([P, SPAD], F32, tag="S", name=f"{pfx}c0b")
            big = [sp0[:, 0:512], sp0[:, 512:1024],
                   sp1[:, 0:512], sp1[:, 512:1024]]
            regions = {}
            for et in range(NET):
                regions[(et, 0)] = big[et]
            if len(chunks) > 1:
                cw1 = chunks[1][1]
                sp2 = spsum.tile([P, SPAD], F32, tag="S", name=f"{pfx}c1a")
                pp0 = ppsum.tile([P, 512], F32, tag="proj", name=f"{pfx}c1b")
                pp1 = ppsum.tile([P, 512], F32, tag="proj", name=f"{pfx}c1c")
                small = [sp2[:, 0:cw1], sp2[:, 512:512 + cw1],
                         pp0[:, 0:cw1], pp1[:, 0:cw1]]
                for et in range(NET):
                    regions[(et, 1)] = small[et]
            for dt_ in range(NDT):
                for et in range(NET):
                    for ci, (c0, cw) in enumerate(chunks):
                        nc.tensor.matmul(
                            regions[(et, ci)],
                            lhsT=w_sb[dt_][:, et * P:(et + 1) * P],
                            rhs=xin[dt_][:, c0:c0 + cw],
                            start=(dt_ == 0),
                            stop=(dt_ == NDT - 1),
                        )
            # chunk-major epilogues: S block 0 needs cols 0:128 of every et
            # tile, which chunk 0 covers -- drain those four chains first
            n = 0
            for ci, (c0, cw) in enumerate(chunks):
                for et in range(NET):
                    relu_epilogue(regions[(et, ci)], bias_sb, out_tiles,
                                  et, c0, cw, on_dve=(n % 2 == 1))
                    n += 1

        def mask_add(kraw, mask_sb, b):
            kTm = [actpool.tile([P, NKP], BF16, tag=f"kTm{et}",
                                name=f"kTm{et}_{b}")
                   for et in range(NET)]
            for et in range(NET):
                # split across gpsimd and vector so neither gates the S phase
                eng = nc.gpsimd if et % 2 == 0 else nc.vector
                eng.tensor_add(kTm[et][:], kraw[et][:], mask_sb[:])
            return kTm

        def s_stats(rs, pad_sb):
            # row-sum -> subtract pad-column contribution -> reciprocal
            rsv = stpool.tile([P, 1], F32, tag="rsv")
            nc.vector.tensor_tensor(
                out=rsv[:], in0=rs[:], in1=pad_sb[:],
                op=mybir.AluOpType.subtract,
            )
            rc = stpool.tile([P, 1], F32, tag="recip")
            nc.vector.reciprocal(out=rc[:], in_=rsv[:])
            return rc

        def s_block(b, ib, qTt, kTm, pad_sb):
            sp = spsum.tile([P, SPAD], F32, tag="S")
            for (c0, cw) in kchunks:
                for et in range(NET):
                    nc.tensor.matmul(
                        sp[:, c0:c0 + cw],
                        lhsT=qTt[et][:, ib * P:(ib + 1) * P],
                        rhs=kTm[et][:, c0:c0 + cw],
                        start=(et == 0),
                        stop=(et == NET - 1),
                    )
            ex = epool.tile([P, NKP], BF16, tag="exp")
            rs = stpool.tile([P, 1], F32, tag="rowsum")
            nc.scalar.activation(
                out=ex[:], in_=sp[:, 0:NKP], func=AF.Exp, scale=SCALE,
                accum_out=rs[:],
            )
            rc = s_stats(rs, pad_sb)
            po = opool.tile([P, NKP], F32, tag="po")
            nc.vector.tensor_scalar(
                out=po[:], in0=ex[:], scalar1=rc[:], scalar2=None,
                op0=mybir.AluOpType.mult,
            )
            # alternate store queues so the output backlog drains 2x faster
            # (sync, not scalar: scalar's ACT must not stall behind DMA issue)
            eng = nc.gpsimd if ib % 2 == 0 else nc.sync
            eng.dma_start(out=out[b, ib * P:(ib + 1) * P, :], in_=po[:])

        def s_block_final(b, ib, qTt, kTm, pad_sb):
            # Last block of the kernel: chunk-major matmuls into separate
            # 1-bank psums + a fully split epilogue so the first chunk's
            # exp/mul/store overlap the second chunk's matmuls and exp --
            # shortening the serial tail after the last MM.
            nch = len(kchunks)
            sps, rss, exs = [], [], []
            for ci, (c0, cw) in enumerate(kchunks):
                sps.append(ppsum.tile([P, 512], F32, tag="proj",
                                      name=f"fsp{ci}"))
                rss.append(stpool.tile([P, 1], F32, tag=f"rowsum{ci}",
                                       name=f"frs{ci}"))
                exs.append(epool.tile([P, cw], BF16, tag=f"fex{ci}",
                                      name=f"fex{ci}"))
            for ci, (c0, cw) in enumerate(kchunks):
                for et in range(NET):
                    nc.tensor.matmul(
                        sps[ci][:, 0:cw],
                        lhsT=qTt[et][:, ib * P:(ib + 1) * P],
                        rhs=kTm[et][:, c0:c0 + cw],
                        start=(et == 0),
                        stop=(et == NET - 1),
                    )
                nc.scalar.activation(
                    out=exs[ci][:], in_=sps[ci][:, 0:cw],
                    func=AF.Exp, scale=SCALE, accum_out=rss[ci][:],
                )
            rs = rss[0]
            for ci in range(1, nch):
                rst = stpool.tile([P, 1], F32, tag="rowsumt", name=f"frt{ci}")
                nc.vector.tensor_tensor(
                    out=rst[:], in0=rs[:], in1=rss[ci][:],
                    op=mybir.AluOpType.add)
                rs = rst
            rc = s_stats(rs, pad_sb)
            for ci, (c0, cw) in enumerate(kchunks):
                poh = opool.tile([P, cw], F32, tag=f"fpo{ci}", name=f"fpo{ci}")
                nc.vector.tensor_scalar(
                    out=poh[:], in0=exs[ci][:],
                    scalar1=rc[:], scalar2=None,
                    op0=mybir.AluOpType.mult,
                )
                eng = nc.gpsimd if ci % 2 == 0 else nc.sync
                eng.dma_start(
                    out=out[b, ib * P:(ib + 1) * P, c0:c0 + cw],
                    in_=poh[:],
                )

        def s_phase(b, qTt, kTm, pad_sb):
            for ib in range(NQB):
                if b == BL - 1 and ib == NQB - 1:
                    s_block_final(b, ib, qTt, kTm, pad_sb)
                else:
                    s_block(b, ib, qTt, kTm, pad_sb)

        cur = load_inputs(0)
        for b in range(BL):
            xk, xq, pad_sb, mask_sb = cur
            if use_mask:
                ktag = "kraw"
            else:
                ktag = "kTm"
            kraw = [actpool.tile([P, NKP], BF16, tag=f"{ktag}{et}",
                                 name=f"{ktag}{et}_{b}")
                    for et in range(NET)]
            if b == 0:
                proj_coldstart(xk, wk_sb, bk_sb, kraw, kchunks, pfx="coldk")
            else:
                proj(xk, wk_sb, bk_sb, kraw, kchunks)
            kTm = mask_add(kraw, mask_sb, b) if use_mask else kraw
            qTt = [actpool.tile([P, NQP], BF16, tag=f"qT{et}",
                                name=f"qT{et}_{b}")
                   for et in range(NET)]
            if b == 0:
                proj_coldstart(xq, wq_sb, bq_sb, qTt, qchunks, pfx="coldq")
            else:
                proj(xq, wq_sb, bq_sb, qTt, qchunks)
            if b + 1 < BL:
                cur = load_inputs(b + 1)
            s_phase(b, qTt, kTm, pad_sb)


def _build(NQP, NKP, use_mask):
    nc = bacc.Bacc(
        "TRN2",
        target_bir_lowering=False,
        debug=False,
        enable_asserts=False,
        num_devices=NCORES,
    )
    qT = nc.dram_tensor("qT", [BL, D, NQP], BF16, kind="ExternalInput").ap()
    kT = nc.dram_tensor("kT", [BL, D, NKP], BF16, kind="ExternalInput").ap()
    Wq = nc.dram_tensor("Wq", [D, D], BF16, kind="ExternalInput").ap()
    Wk = nc.dram_tensor("Wk", [D, D], BF16, kind="ExternalInput").ap()
    bq = nc.dram_tensor("bq", [P, NET], F32, kind="ExternalInput").ap()
    bk = nc.dram_tensor("bk", [P, NET], F32, kind="ExternalInput").ap()
    padc = nc.dram_tensor("padc", [BL, P, 1], F32, kind="ExternalInput").ap()
    maskc = None
    if use_mask:
        maskc = nc.dram_tensor(
            "maskc", [BL, P, NKP], BF16, kind="ExternalInput").ap()
    out = nc.dram_tensor("out", [BL, NQP, NKP], F32, kind="ExternalOutput").ap()

    with tile.TileContext(nc) as tc:
        _body(tc, qT, kT, Wq, Wk, bq, bk, padc, maskc, out, NQP, NKP)
    nc.compile()
    return nc


def _get_nc(NQP, NKP, use_mask):
    key = (NQP, NKP, use_mask)
    if key not in _CACHE:
        _CACHE[key] = _build(*key)
    return _CACHE[key]


def _pad128(n):
    return max(P, ((n + P - 1) // P) * P)


def _prep(query, key, query_mask, key_mask, Wq, bq, Wk, bk):
    bf = ml_dtypes.bfloat16
    query = np.asarray(query, dtype=np.float32)
    key = np.asarray(key, dtype=np.float32)
    qmask = np.asarray(query_mask) != 0
    kmask = np.asarray(key_mask) != 0
    qidx = [np.nonzero(qmask[g])[0] for g in range(B)]
    kidx = [np.nonzero(kmask[g])[0] for g in range(B)]
    NQP = _pad128(max(len(i) for i in qidx))
    NKP = _pad128(max(len(i) for i in kidx))
    use_mask = bool(np.any(np.asarray(bk, dtype=np.float32) != 0.0))

    Wq_b = np.asarray(Wq, dtype=np.float32).astype(bf)
    Wk_b = np.asarray(Wk, dtype=np.float32).astype(bf)
    # bias for feature e lives at partition e%128, column e//128
    bq_t = np.asarray(bq, dtype=np.float32).reshape(NET, P).T.copy()
    bk_t = np.asarray(bk, dtype=np.float32).reshape(NET, P).T.copy()

    in_maps = []
    for c in range(NCORES):
        qTc = np.zeros((BL, D, NQP), dtype=bf)
        kTc = np.zeros((BL, D, NKP), dtype=bf)
        padc = np.zeros((BL, P, 1), dtype=np.float32)
        imap = {"qT": qTc, "kT": kTc, "Wq": Wq_b, "Wk": Wk_b,
                "bq": bq_t, "bk": bk_t, "padc": padc}
        if use_mask:
            mk = np.zeros((BL, P, NKP), dtype=bf)
            imap["maskc"] = mk
        for b in range(BL):
            g = c * BL + b
            qi, ki = qidx[g], kidx[g]
            qTc[b, :, :len(qi)] = query[g][qi].T.astype(bf)
            kTc[b, :, :len(ki)] = key[g][ki].T.astype(bf)
            if use_mask:
                imap["maskc"][b, :, len(ki):] = bf(MASKC)
            else:
                padc[b, :, 0] = float(NKP - len(ki))
        in_maps.append(imap)
    return in_maps, qidx, kidx, NQP, NKP, use_mask


def run(query, key, query_mask, key_mask, Wq, bq, Wk, bk, **kwargs):
    """Run on hardware; returns (output, BassKernelResults)."""
    in_maps, qidx, kidx, NQP, NKP, use_mask = _prep(
        query, key, query_mask, key_mask, Wq, bq, Wk, bk)
    nc = _get_nc(NQP, NKP, use_mask)
    res = run_bass_kernel_spmd(nc, in_maps, core_ids=list(range(NCORES)),
                               **kwargs)
    full = np.zeros((B, LQ, LK), dtype=np.float32)
    for c in range(NCORES):
        oc = res.results[c]["out"]
        for b in range(BL):
            g = c * BL + b
            qi, ki = qidx[g], kidx[g]
            full[g][np.ix_(qi, ki)] = oc[b][:len(qi), :len(ki)]
    return full, res


def kernel(query, key, query_mask, key_mask, Wq, bq, Wk, bk):
    full, _ = run(query, key, query_mask, key_mask, Wq, bq, Wk, bk)
    return full


# revision 5
# speedup vs baseline: 1.6522x; 1.1883x over previous
"""Masked attention-weight kernel (dense_transformer) for 8 TRN2 NeuronCores.

Computes, for inputs query/key [32,1024,512] f32, masks [32,1024] i32:
    q = relu(query @ Wq + bq); k = relu(key @ Wk + bk)
    w = softmax((q @ k^T)/sqrt(512) + key_mask_additive) * query_mask
Output: [32, 1024, 1024] f32.

Strategy: data-parallel over batch (4 batches/core, no collectives) PLUS
host-side mask compaction.  Masked key columns have weight exactly 0 in the
reference (exp(-1e9) underflows) and masked query rows are zeroed, so the
host gathers only the valid ~512 query rows / key columns per batch, pads
them to a fixed NQP/NKP (multiple of 128, 640 for this data), and the device
runs dense attention on the compacted [NQP, NKP] problem -- ~2.2x fewer
matmul cycles than the full [1024,1024].  The host scatters the compact
output back into a zero-filled full-size array.

Padded key columns are all-zero inputs, so (with zero bias -- true for this
problem) their projected features are 0, their logits are 0, and each
contributes exp(0)=1 to the softmax row-sum; the device subtracts the
host-provided pad count from the row-sum before taking the reciprocal.
If the key bias were nonzero the host instead ships an additive -1e4
column mask applied to the projected k (use_mask variant).

Per-core pipeline, per batch (all matmuls bf16 with f32 PSUM):
  1. kTm[e,j] = relu(Wk.T @ keyT + bk): PE matmuls in (512,128) psum-bank
     chunks -> relu+bias epilogue alternating ACT/DVE.
  2. qT[e,i] likewise.  Batch 0 runs both projections dt-major across 8
     open psum chains so the PE consumes (w_dt, x_dt) DMA pairs in arrival
     order at cold start.
  3. Per 128-row block: S = qT.T @ kTm (PE), ACT exp with fused row-sum,
     DVE pad-correction + reciprocal, DVE scale, DMA out (stores alternate
     between the gpsimd and sync queues).
"""

import sys

sys.path.insert(0, "/opt/trn_rl_repo")

import numpy as np
import ml_dtypes
from contextlib import ExitStack

import concourse.tile as tile
from concourse import bacc, mybir
from concourse.bass_utils import run_bass_kernel_spmd

P = 128
B, LQ, LK, D = 32, 1024, 1024, 512
NCORES = 8
BL = B // NCORES          # batches per core
NDT = D // P              # contraction tiles for projections
NET = D // P              # output-feature tiles (= S contraction tiles)
SCALE = float(1.0 / np.sqrt(D))
MASKC = -1.0e4

F32 = mybir.dt.float32
BF16 = mybir.dt.bfloat16
AF = mybir.ActivationFunctionType

_CACHE = {}


def _chunks(width):
    """Split a free width into psum-bank-aligned chunks (<=512 each)."""
    out, c0 = [], 0
    while c0 < width:
        cw = min(512, width - c0)
        out.append((c0, cw))
        c0 += cw
    return out


def _body(tc, qT, kT, Wq, Wk, bq, bk, padc, maskc, out, NQP, NKP):
    nc = tc.nc
    NQB = NQP // P            # 128-row S blocks per batch
    SPAD = ((NKP + 511) // 512) * 512   # psum tile width (bank aligned)
    kchunks = _chunks(NKP)
    qchunks = _chunks(NQP)
    use_mask = maskc is not None
    with ExitStack() as ctx:
        consts = ctx.enter_context(tc.tile_pool(name="consts", bufs=1))
        wpool = ctx.enter_context(tc.tile_pool(name="w", bufs=1))
        inpool = ctx.enter_context(tc.tile_pool(name="inp", bufs=2))
        actpool = ctx.enter_context(tc.tile_pool(name="act", bufs=2))
        mpool = ctx.enter_context(tc.tile_pool(name="mask", bufs=2))
        epool = ctx.enter_context(tc.tile_pool(name="exp", bufs=3))
        opool = ctx.enter_context(tc.tile_pool(name="pout", bufs=3))
        stpool = ctx.enter_context(tc.tile_pool(name="stat", bufs=6))
        ppsum = ctx.enter_context(tc.tile_pool(name="ppsum", bufs=3, space="PSUM"))
        spsum = ctx.enter_context(tc.tile_pool(name="spsum", bufs=2, space="PSUM"))
        cpsum = ctx.enter_context(tc.tile_pool(name="cpsum", bufs=1, space="PSUM"))

        # Weights on the scalar DMA queue, inputs on sync, small tensors on
        # gpsimd -- three queues pull concurrently at cold start.
        wk_sb = [wpool.tile([P, D], BF16, tag=f"wk{dt_}", name=f"wk{dt_}")
                 for dt_ in range(NDT)]
        wq_sb = [wpool.tile([P, D], BF16, tag=f"wq{dt_}", name=f"wq{dt_}")
                 for dt_ in range(NDT)]
        for dt_ in range(NDT):
            nc.scalar.dma_start(
                out=wk_sb[dt_][:], in_=Wk[dt_ * P:(dt_ + 1) * P, :])
        for dt_ in range(NDT):
            nc.scalar.dma_start(
                out=wq_sb[dt_][:], in_=Wq[dt_ * P:(dt_ + 1) * P, :])

        bk_sb = consts.tile([P, NET], F32)
        nc.gpsimd.dma_start(out=bk_sb[:], in_=bk[:])
        bq_sb = consts.tile([P, NET], F32)
        nc.gpsimd.dma_start(out=bq_sb[:], in_=bq[:])

        # PE warmup: 8 dummy matmuls (~3.4us of cold PE busy, exactly one
        # HAM activity window) on scratch tiles while the first input DMAs
        # are in flight, so the clock-gate reaches K=8/8 just before real
        # matmuls start. Results are never read.
        warm_in = consts.tile([P, 512], BF16, name="warm_in")
        nc.vector.memset(warm_in[:], 0.0)
        warm_ps = ppsum.tile([P, 512], F32, tag="proj", name="warm_ps")
        for _ in range(8):
            nc.tensor.matmul(
                warm_ps[:], lhsT=warm_in[:, 0:P], rhs=warm_in[:],
                start=True, stop=True,
            )

        def load_inputs(b):
            xk, xq = [], []
            for dt_ in range(NDT):
                t = inpool.tile([P, NKP], BF16, tag=f"xk{dt_}")
                if b == 0:
                    # split so chunk-0 matmul deps land sooner at cold start
                    for (c0, cw) in kchunks:
                        nc.sync.dma_start(
                            out=t[:, c0:c0 + cw],
                            in_=kT[b, dt_ * P:(dt_ + 1) * P, c0:c0 + cw])
                else:
                    nc.sync.dma_start(
                        out=t[:], in_=kT[b, dt_ * P:(dt_ + 1) * P, :])
                xk.append(t)
            for dt_ in range(NDT):
                t = inpool.tile([P, NQP], BF16, tag=f"xq{dt_}")
                if b == 0:
                    for (c0, cw) in qchunks:
                        nc.sync.dma_start(
                            out=t[:, c0:c0 + cw],
                            in_=qT[b, dt_ * P:(dt_ + 1) * P, c0:c0 + cw])
                else:
                    nc.sync.dma_start(
                        out=t[:], in_=qT[b, dt_ * P:(dt_ + 1) * P, :])
                xq.append(t)
            pad_sb = mpool.tile([P, 1], F32, tag="padc")
            nc.gpsimd.dma_start(out=pad_sb[:], in_=padc[b])
            mask_sb = None
            if use_mask:
                mask_sb = mpool.tile([P, NKP], BF16, tag="maskc")
                nc.gpsimd.dma_start(out=mask_sb[:], in_=maskc[b])
            return xk, xq, pad_sb, mask_sb

        def relu_epilogue(ps, bias_sb, out_tiles, et, c0, cw, on_dve):
            if on_dve:
                # (psum + bias) max 0 -- exact relu+bias as one DVE op
                nc.vector.tensor_scalar(
                    out=out_tiles[et][:, c0:c0 + cw],
                    in0=ps,
                    scalar1=bias_sb[:, et:et + 1],
                    scalar2=0.0,
                    op0=mybir.AluOpType.add,
                    op1=mybir.AluOpType.max,
                )
            else:
                nc.scalar.activation(
                    out=out_tiles[et][:, c0:c0 + cw],
                    in_=ps,
                    func=AF.Relu,
                    bias=bias_sb[:, et:et + 1],
                    scale=1.0,
                )

        def proj(xin, w_sb, bias_sb, out_tiles, chunks):
            # out_tiles[et] = relu(W[:, et].T @ x + b)
            n = 0
            for et in range(NET):
                for (c0, cw) in chunks:
                    ps = ppsum.tile([P, 512], F32, tag="proj")
                    for dt_ in range(NDT):
                        nc.tensor.matmul(
                            ps[:, 0:cw],
                            lhsT=w_sb[dt_][:, et * P:(et + 1) * P],
                            rhs=xin[dt_][:, c0:c0 + cw],
                            start=(dt_ == 0),
                            stop=(dt_ == NDT - 1),
                        )
                    relu_epilogue(ps[:, 0:cw], bias_sb, out_tiles, et, c0, cw,
                                  on_dve=(n % 2 == 1))
                    n += 1

        def proj_coldstart(xin, w_sb, bias_sb, out_tiles, chunks, pfx):
            # Batch-0 projections: dt-major order so the PE consumes
            # (w_dt, x_dt) DMA pairs in arrival order; all NET*len(chunks)
            # accumulation chains are open at once, borrowing the (still
            # idle) S-phase psum pool.  Chain -> single-bank psum region:
            #   chunk0 (512 wide) x4 et -> spsum tiles 0,1 (two banks each)
            #   chunk1 (<=128)    x4 et -> ppsum x3 + cpsum
            sp0 = spsum.tile([P, SPAD], F32, tag="S", name=f"{pfx}c0a")
            sp1 = spsum.tile([P, SPAD], F32, tag="S", name=f"{pfx}c0b")
            big = [sp0[:, 0:512], sp0[:, 512:1024],
                   sp1[:, 0:512], sp1[:, 512:1024]]
            regions = {}
            for et in range(NET):
                regions[(et, 0)] = big[et]
            if len(chunks) > 1:
                cw1 = chunks[1][1]
                pps = [ppsum.tile([P, 512], F32, tag="proj",
                                  name=f"{pfx}c1{i}") for i in range(3)]
                pps.append(cpsum.tile([P, 512], F32, tag="cold",
                                      name=f"{pfx}c1x"))
                for et in range(NET):
                    regions[(et, 1)] = pps[et][:, 0:cw1]
            for dt_ in range(NDT):
                for et in range(NET):
                    for ci, (c0, cw) in enumerate(chunks):
                        nc.tensor.matmul(
                            regions[(et, ci)],
                            lhsT=w_sb[dt_][:, et * P:(et + 1) * P],
                            rhs=xin[dt_][:, c0:c0 + cw],
                            start=(dt_ == 0),
                            stop=(dt_ == NDT - 1),
                        )
            # chunk-major epilogues: S block 0 needs cols 0:128 of every et
            # tile, which chunk 0 covers -- drain those four chains first
            n = 0
            for ci, (c0, cw) in enumerate(chunks):
                for et in range(NET):
                    relu_epilogue(regions[(et, ci)], bias_sb, out_tiles,
                                  et, c0, cw, on_dve=(n % 2 == 1))
                    n += 1

        def mask_add(kraw, mask_sb, b):
            kTm = [actpool.tile([P, NKP], BF16, tag=f"kTm{et}",
                                name=f"kTm{et}_{b}")
                   for et in range(NET)]
            for et in range(NET):
                # split across gpsimd and vector so neither gates the S phase
                eng = nc.gpsimd if et % 2 == 0 else nc.vector
                eng.tensor_add(kTm[et][:], kraw[et][:], mask_sb[:])
            return kTm

        def s_stats(rs, pad_sb):
            # row-sum -> subtract pad-column contribution -> reciprocal
            rsv = stpool.tile([P, 1], F32, tag="rsv")
            nc.vector.tensor_tensor(
                out=rsv[:], in0=rs[:], in1=pad_sb[:],
                op=mybir.AluOpType.subtract,
            )
            rc = stpool.tile([P, 1], F32, tag="recip")
            nc.vector.reciprocal(out=rc[:], in_=rsv[:])
            return rc

        def s_block(b, ib, qTt, kTm, pad_sb):
            sp = spsum.tile([P, SPAD], F32, tag="S")
            for (c0, cw) in kchunks:
                for et in range(NET):
                    nc.tensor.matmul(
                        sp[:, c0:c0 + cw],
                        lhsT=qTt[et][:, ib * P:(ib + 1) * P],
                        rhs=kTm[et][:, c0:c0 + cw],
                        start=(et == 0),
                        stop=(et == NET - 1),
                    )
            ex = epool.tile([P, NKP], BF16, tag="exp")
            rs = stpool.tile([P, 1], F32, tag="rowsum")
            nc.scalar.activation(
                out=ex[:], in_=sp[:, 0:NKP], func=AF.Exp, scale=SCALE,
                accum_out=rs[:],
            )
            rc = s_stats(rs, pad_sb)
            po = opool.tile([P, NKP], BF16, tag="po")
            nc.vector.tensor_scalar(
                out=po[:], in0=ex[:], scalar1=rc[:], scalar2=None,
                op0=mybir.AluOpType.mult,
            )
            # alternate store queues so the output backlog drains 2x faster
            # (sync, not scalar: scalar's ACT must not stall behind DMA issue)
            eng = nc.gpsimd if ib % 2 == 0 else nc.sync
            eng.dma_start(out=out[b, ib * P:(ib + 1) * P, :], in_=po[:])

        def s_block_final(b, ib, qTt, kTm, pad_sb):
            # Last block of the kernel: chunk-major matmuls into separate
            # 1-bank psums + a fully split epilogue so the first chunk's
            # exp/mul/store overlap the second chunk's matmuls and exp --
            # shortening the serial tail after the last MM.
            nch = len(kchunks)
            sps, rss, exs = [], [], []
            for ci, (c0, cw) in enumerate(kchunks):
                sps.append(ppsum.tile([P, 512], F32, tag="proj",
                                      name=f"fsp{ci}"))
                rss.append(stpool.tile([P, 1], F32, tag=f"rowsum{ci}",
                                       name=f"frs{ci}"))
                exs.append(epool.tile([P, cw], BF16, tag=f"fex{ci}",
                                      name=f"fex{ci}"))
            for ci, (c0, cw) in enumerate(kchunks):
                for et in range(NET):
                    nc.tensor.matmul(
                        sps[ci][:, 0:cw],
                        lhsT=qTt[et][:, ib * P:(ib + 1) * P],
                        rhs=kTm[et][:, c0:c0 + cw],
                        start=(et == 0),
                        stop=(et == NET - 1),
                    )
                nc.scalar.activation(
                    out=exs[ci][:], in_=sps[ci][:, 0:cw],
                    func=AF.Exp, scale=SCALE, accum_out=rss[ci][:],
                )
            rs = rss[0]
            for ci in range(1, nch):
                rst = stpool.tile([P, 1], F32, tag="rowsumt", name=f"frt{ci}")
                nc.vector.tensor_tensor(
                    out=rst[:], in0=rs[:], in1=rss[ci][:],
                    op=mybir.AluOpType.add)
                rs = rst
            rc = s_stats(rs, pad_sb)
            for ci, (c0, cw) in enumerate(kchunks):
                poh = opool.tile([P, cw], BF16, tag=f"fpo{ci}", name=f"fpo{ci}")
                nc.vector.tensor_scalar(
                    out=poh[:], in0=exs[ci][:],
                    scalar1=rc[:], scalar2=None,
                    op0=mybir.AluOpType.mult,
                )
                eng = nc.gpsimd if ci % 2 == 0 else nc.sync
                eng.dma_start(
                    out=out[b, ib * P:(ib + 1) * P, c0:c0 + cw],
                    in_=poh[:],
                )

        def s_phase(b, qTt, kTm, pad_sb):
            for ib in range(NQB):
                if b == BL - 1 and ib == NQB - 1:
                    s_block_final(b, ib, qTt, kTm, pad_sb)
                else:
                    s_block(b, ib, qTt, kTm, pad_sb)

        cur = load_inputs(0)
        for b in range(BL):
            xk, xq, pad_sb, mask_sb = cur
            if use_mask:
                ktag = "kraw"
            else:
                ktag = "kTm"
            kraw = [actpool.tile([P, NKP], BF16, tag=f"{ktag}{et}",
                                 name=f"{ktag}{et}_{b}")
                    for et in range(NET)]
            if b == 0:
                proj_coldstart(xk, wk_sb, bk_sb, kraw, kchunks, pfx="coldk")
            else:
                proj(xk, wk_sb, bk_sb, kraw, kchunks)
            kTm = mask_add(kraw, mask_sb, b) if use_mask else kraw
            qTt = [actpool.tile([P, NQP], BF16, tag=f"qT{et}",
                                name=f"qT{et}_{b}")
                   for et in range(NET)]
            if b == 0:
                proj_coldstart(xq, wq_sb, bq_sb, qTt, qchunks, pfx="coldq")
            else:
                proj(xq, wq_sb, bq_sb, qTt, qchunks)
            if b + 1 < BL:
                cur = load_inputs(b + 1)
            s_phase(b, qTt, kTm, pad_sb)


def _build(NQP, NKP, use_mask):
    nc = bacc.Bacc(
        "TRN2",
        target_bir_lowering=False,
        debug=False,
        enable_asserts=False,
        num_devices=NCORES,
    )
    qT = nc.dram_tensor("qT", [BL, D, NQP], BF16, kind="ExternalInput").ap()
    kT = nc.dram_tensor("kT", [BL, D, NKP], BF16, kind="ExternalInput").ap()
    Wq = nc.dram_tensor("Wq", [D, D], BF16, kind="ExternalInput").ap()
    Wk = nc.dram_tensor("Wk", [D, D], BF16, kind="ExternalInput").ap()
    bq = nc.dram_tensor("bq", [P, NET], F32, kind="ExternalInput").ap()
    bk = nc.dram_tensor("bk", [P, NET], F32, kind="ExternalInput").ap()
    padc = nc.dram_tensor("padc", [BL, P, 1], F32, kind="ExternalInput").ap()
    maskc = None
    if use_mask:
        maskc = nc.dram_tensor(
            "maskc", [BL, P, NKP], BF16, kind="ExternalInput").ap()
    out = nc.dram_tensor("out", [BL, NQP, NKP], BF16, kind="ExternalOutput").ap()

    with tile.TileContext(nc) as tc:
        _body(tc, qT, kT, Wq, Wk, bq, bk, padc, maskc, out, NQP, NKP)
    nc.compile()
    return nc


def _get_nc(NQP, NKP, use_mask):
    key = (NQP, NKP, use_mask)
    if key not in _CACHE:
        _CACHE[key] = _build(*key)
    return _CACHE[key]


def _pad128(n):
    return max(P, ((n + P - 1) // P) * P)


def _prep(query, key, query_mask, key_mask, Wq, bq, Wk, bk):
    bf = ml_dtypes.bfloat16
    query = np.asarray(query, dtype=np.float32)
    key = np.asarray(key, dtype=np.float32)
    qmask = np.asarray(query_mask) != 0
    kmask = np.asarray(key_mask) != 0
    qidx = [np.nonzero(qmask[g])[0] for g in range(B)]
    kidx = [np.nonzero(kmask[g])[0] for g in range(B)]
    NQP = _pad128(max(len(i) for i in qidx))
    NKP = _pad128(max(len(i) for i in kidx))
    use_mask = bool(np.any(np.asarray(bk, dtype=np.float32) != 0.0))

    Wq_b = np.asarray(Wq, dtype=np.float32).astype(bf)
    Wk_b = np.asarray(Wk, dtype=np.float32).astype(bf)
    # bias for feature e lives at partition e%128, column e//128
    bq_t = np.asarray(bq, dtype=np.float32).reshape(NET, P).T.copy()
    bk_t = np.asarray(bk, dtype=np.float32).reshape(NET, P).T.copy()

    in_maps = []
    for c in range(NCORES):
        qTc = np.zeros((BL, D, NQP), dtype=bf)
        kTc = np.zeros((BL, D, NKP), dtype=bf)
        padc = np.zeros((BL, P, 1), dtype=np.float32)
        imap = {"qT": qTc, "kT": kTc, "Wq": Wq_b, "Wk": Wk_b,
                "bq": bq_t, "bk": bk_t, "padc": padc}
        if use_mask:
            mk = np.zeros((BL, P, NKP), dtype=bf)
            imap["maskc"] = mk
        for b in range(BL):
            g = c * BL + b
            qi, ki = qidx[g], kidx[g]
            qTc[b, :, :len(qi)] = query[g][qi].T.astype(bf)
            kTc[b, :, :len(ki)] = key[g][ki].T.astype(bf)
            if use_mask:
                imap["maskc"][b, :, len(ki):] = bf(MASKC)
            else:
                padc[b, :, 0] = float(NKP - len(ki))
        in_maps.append(imap)
    return in_maps, qidx, kidx, NQP, NKP, use_mask


def run(query, key, query_mask, key_mask, Wq, bq, Wk, bk, **kwargs):
    """Run on hardware; returns (output, BassKernelResults)."""
    in_maps, qidx, kidx, NQP, NKP, use_mask = _prep(
        query, key, query_mask, key_mask, Wq, bq, Wk, bk)
    nc = _get_nc(NQP, NKP, use_mask)
    res = run_bass_kernel_spmd(nc, in_maps, core_ids=list(range(NCORES)),
                               **kwargs)
    full = np.zeros((B, LQ, LK), dtype=np.float32)
    for c in range(NCORES):
        oc = res.results[c]["out"]
        for b in range(BL):
            g = c * BL + b
            qi, ki = qidx[g], kidx[g]
            full[g][np.ix_(qi, ki)] = oc[b][:len(qi), :len(ki)].astype(np.float32)
    return full, res


def kernel(query, key, query_mask, key_mask, Wq, bq, Wk, bk):
    full, _ = run(query, key, query_mask, key_mask, Wq, bq, Wk, bk)
    return full


# revision 6
# speedup vs baseline: 1.7061x; 1.0326x over previous
"""Masked attention-weight kernel (dense_transformer) for 8 TRN2 NeuronCores.

Computes, for inputs query/key [32,1024,512] f32, masks [32,1024] i32:
    q = relu(query @ Wq + bq); k = relu(key @ Wk + bk)
    w = softmax((q @ k^T)/sqrt(512) + key_mask_additive) * query_mask
Output: [32, 1024, 1024] f32.

Strategy: data-parallel over batch (4 batches/core, no collectives) PLUS
host-side mask compaction.  Masked key columns have weight exactly 0 in the
reference (exp(-1e9) underflows) and masked query rows are zeroed, so the
host gathers only the valid ~512 query rows / key columns per batch, pads
them to a fixed NQP/NKP (multiple of 128, 640 for this data), and the device
runs dense attention on the compacted [NQP, NKP] problem -- ~2.2x fewer
matmul cycles than the full [1024,1024].  The host scatters the compact
output back into a zero-filled full-size array.

Padded key columns are all-zero inputs, so (with zero bias -- true for this
problem) their projected features are 0, their logits are 0, and each
contributes exp(0)=1 to the softmax row-sum; the device subtracts the
host-provided pad count from the row-sum before taking the reciprocal.
If the key bias were nonzero the host instead ships an additive -1e4
column mask applied to the projected k (use_mask variant).

Per-core pipeline, per batch (all matmuls bf16 with f32 PSUM):
  1. kTm[e,j] = relu(Wk.T @ keyT + bk): PE matmuls in (512,128) psum-bank
     chunks -> relu+bias epilogue alternating ACT/DVE.
  2. qT[e,i] likewise.  Batch 0 runs both projections dt-major across 8
     open psum chains so the PE consumes (w_dt, x_dt) DMA pairs in arrival
     order at cold start.
  3. Per 128-row block: S = qT.T @ kTm (PE), ACT exp with fused row-sum,
     DVE pad-correction + reciprocal, DVE scale, DMA out (stores alternate
     between the gpsimd and sync queues).
"""

import sys

sys.path.insert(0, "/opt/trn_rl_repo")

import numpy as np
import ml_dtypes
from contextlib import ExitStack

import concourse.tile as tile
from concourse import bacc, mybir
from concourse.bass_utils import run_bass_kernel_spmd

P = 128
B, LQ, LK, D = 32, 1024, 1024, 512
NCORES = 8
BL = B // NCORES          # batches per core
NDT = D // P              # contraction tiles for projections
NET = D // P              # output-feature tiles (= S contraction tiles)
SCALE = float(1.0 / np.sqrt(D))
MASKC = -1.0e4

F32 = mybir.dt.float32
BF16 = mybir.dt.bfloat16
FP8 = mybir.dt.float8e4
AF = mybir.ActivationFunctionType

_CACHE = {}


def _chunks(width):
    """Split a free width into psum-bank-aligned chunks (<=512 each)."""
    out, c0 = [], 0
    while c0 < width:
        cw = min(512, width - c0)
        out.append((c0, cw))
        c0 += cw
    return out


def _body(tc, qT, kT, Wq, Wk, bq, bk, padc, maskc, out, NQP, NKP):
    nc = tc.nc
    s_fp8 = maskc is None      # fp8 DoubleRow S-matmul (skip if mask-add)
    NQB = NQP // P            # 128-row S blocks per batch
    SPAD = ((NKP + 511) // 512) * 512   # psum tile width (bank aligned)
    kchunks = _chunks(NKP)
    qchunks = _chunks(NQP)
    use_mask = maskc is not None
    with ExitStack() as ctx:
        consts = ctx.enter_context(tc.tile_pool(name="consts", bufs=1))
        wpool = ctx.enter_context(tc.tile_pool(name="w", bufs=1))
        inpool = ctx.enter_context(tc.tile_pool(name="inp", bufs=2))
        actpool = ctx.enter_context(tc.tile_pool(name="act", bufs=2))
        mpool = ctx.enter_context(tc.tile_pool(name="mask", bufs=2))
        epool = ctx.enter_context(tc.tile_pool(name="exp", bufs=3))
        opool = ctx.enter_context(tc.tile_pool(name="pout", bufs=3))
        stpool = ctx.enter_context(tc.tile_pool(name="stat", bufs=6))
        ppsum = ctx.enter_context(tc.tile_pool(name="ppsum", bufs=3, space="PSUM"))
        spsum = ctx.enter_context(tc.tile_pool(name="spsum", bufs=2, space="PSUM"))
        cpsum = ctx.enter_context(tc.tile_pool(name="cpsum", bufs=1, space="PSUM"))

        # Weights on the scalar DMA queue, inputs on sync, small tensors on
        # gpsimd -- three queues pull concurrently at cold start.
        wk_sb = [wpool.tile([P, D], BF16, tag=f"wk{dt_}", name=f"wk{dt_}")
                 for dt_ in range(NDT)]
        wq_sb = [wpool.tile([P, D], BF16, tag=f"wq{dt_}", name=f"wq{dt_}")
                 for dt_ in range(NDT)]
        for dt_ in range(NDT):
            nc.scalar.dma_start(
                out=wk_sb[dt_][:], in_=Wk[dt_ * P:(dt_ + 1) * P, :])
        for dt_ in range(NDT):
            nc.scalar.dma_start(
                out=wq_sb[dt_][:], in_=Wq[dt_ * P:(dt_ + 1) * P, :])

        bk_sb = consts.tile([P, NET], F32)
        nc.gpsimd.dma_start(out=bk_sb[:], in_=bk[:])
        bq_sb = consts.tile([P, NET], F32)
        nc.gpsimd.dma_start(out=bq_sb[:], in_=bq[:])

        # PE warmup: 8 dummy matmuls (~3.4us of cold PE busy, exactly one
        # HAM activity window) on scratch tiles while the first input DMAs
        # are in flight, so the clock-gate reaches K=8/8 just before real
        # matmuls start. Results are never read.
        warm_in = consts.tile([P, 512], BF16, name="warm_in")
        nc.vector.memset(warm_in[:], 0.0)
        warm_ps = ppsum.tile([P, 512], F32, tag="proj", name="warm_ps")
        for _ in range(8):
            nc.tensor.matmul(
                warm_ps[:], lhsT=warm_in[:, 0:P], rhs=warm_in[:],
                start=True, stop=True,
            )

        def load_inputs(b):
            xk, xq = [], []
            for dt_ in range(NDT):
                t = inpool.tile([P, NKP], BF16, tag=f"xk{dt_}")
                if b == 0:
                    # split so chunk-0 matmul deps land sooner at cold start
                    for (c0, cw) in kchunks:
                        nc.sync.dma_start(
                            out=t[:, c0:c0 + cw],
                            in_=kT[b, dt_ * P:(dt_ + 1) * P, c0:c0 + cw])
                else:
                    nc.sync.dma_start(
                        out=t[:], in_=kT[b, dt_ * P:(dt_ + 1) * P, :])
                xk.append(t)
            for dt_ in range(NDT):
                t = inpool.tile([P, NQP], BF16, tag=f"xq{dt_}")
                if b == 0:
                    for (c0, cw) in qchunks:
                        nc.sync.dma_start(
                            out=t[:, c0:c0 + cw],
                            in_=qT[b, dt_ * P:(dt_ + 1) * P, c0:c0 + cw])
                else:
                    nc.sync.dma_start(
                        out=t[:], in_=qT[b, dt_ * P:(dt_ + 1) * P, :])
                xq.append(t)
            pad_sb = mpool.tile([P, 1], F32, tag="padc")
            nc.gpsimd.dma_start(out=pad_sb[:], in_=padc[b])
            mask_sb = None
            if use_mask:
                mask_sb = mpool.tile([P, NKP], BF16, tag="maskc")
                nc.gpsimd.dma_start(out=mask_sb[:], in_=maskc[b])
            return xk, xq, pad_sb, mask_sb

        def _eslice(out_tiles, et, c0, cw):
            # fp8 mode packs et pairs into [P, 2, N] DoubleRow operand tiles
            if s_fp8:
                return out_tiles[et // 2][:, et % 2, c0:c0 + cw]
            return out_tiles[et][:, c0:c0 + cw]

        def relu_epilogue(ps, bias_sb, out_tiles, et, c0, cw, on_dve):
            if on_dve:
                # (psum + bias) max 0 -- exact relu+bias as one DVE op
                nc.vector.tensor_scalar(
                    out=_eslice(out_tiles, et, c0, cw),
                    in0=ps,
                    scalar1=bias_sb[:, et:et + 1],
                    scalar2=0.0,
                    op0=mybir.AluOpType.add,
                    op1=mybir.AluOpType.max,
                )
            else:
                nc.scalar.activation(
                    out=_eslice(out_tiles, et, c0, cw),
                    in_=ps,
                    func=AF.Relu,
                    bias=bias_sb[:, et:et + 1],
                    scale=1.0,
                )

        def proj(xin, w_sb, bias_sb, out_tiles, chunks):
            # out_tiles[et] = relu(W[:, et].T @ x + b)
            n = 0
            for et in range(NET):
                for (c0, cw) in chunks:
                    ps = ppsum.tile([P, 512], F32, tag="proj")
                    for dt_ in range(NDT):
                        nc.tensor.matmul(
                            ps[:, 0:cw],
                            lhsT=w_sb[dt_][:, et * P:(et + 1) * P],
                            rhs=xin[dt_][:, c0:c0 + cw],
                            start=(dt_ == 0),
                            stop=(dt_ == NDT - 1),
                        )
                    relu_epilogue(ps[:, 0:cw], bias_sb, out_tiles, et, c0, cw,
                                  on_dve=(n % 2 == 1))
                    n += 1

        def proj_coldstart(xin, w_sb, bias_sb, out_tiles, chunks, pfx):
            # Batch-0 projections: dt-major order so the PE consumes
            # (w_dt, x_dt) DMA pairs in arrival order; all NET*len(chunks)
            # accumulation chains are open at once, borrowing the (still
            # idle) S-phase psum pool.  Chain -> single-bank psum region:
            #   chunk0 (512 wide) x4 et -> spsum tiles 0,1 (two banks each)
            #   chunk1 (<=128)    x4 et -> ppsum x3 + cpsum
            sp0 = spsum.tile([P, SPAD], F32, tag="S", name=f"{pfx}c0a")
            sp1 = spsum.tile([P, SPAD], F32, tag="S", name=f"{pfx}c0b")
            big = [sp0[:, 0:512], sp0[:, 512:1024],
                   sp1[:, 0:512], sp1[:, 512:1024]]
            regions = {}
            for et in range(NET):
                regions[(et, 0)] = big[et]
            if len(chunks) > 1:
                cw1 = chunks[1][1]
                pps = [ppsum.tile([P, 512], F32, tag="proj",
                                  name=f"{pfx}c1{i}") for i in range(3)]
                pps.append(cpsum.tile([P, 512], F32, tag="cold",
                                      name=f"{pfx}c1x"))
                for et in range(NET):
                    regions[(et, 1)] = pps[et][:, 0:cw1]
            for dt_ in range(NDT):
                for et in range(NET):
                    for ci, (c0, cw) in enumerate(chunks):
                        nc.tensor.matmul(
                            regions[(et, ci)],
                            lhsT=w_sb[dt_][:, et * P:(et + 1) * P],
                            rhs=xin[dt_][:, c0:c0 + cw],
                            start=(dt_ == 0),
                            stop=(dt_ == NDT - 1),
                        )
            # chunk-major epilogues: S block 0 needs cols 0:128 of every et
            # tile, which chunk 0 covers -- drain those four chains first
            n = 0
            for ci, (c0, cw) in enumerate(chunks):
                for et in range(NET):
                    relu_epilogue(regions[(et, ci)], bias_sb, out_tiles,
                                  et, c0, cw, on_dve=(n % 2 == 1))
                    n += 1

        def mask_add(kraw, mask_sb, b):
            kTm = [actpool.tile([P, NKP], BF16, tag=f"kTm{et}",
                                name=f"kTm{et}_{b}")
                   for et in range(NET)]
            for et in range(NET):
                # split across gpsimd and vector so neither gates the S phase
                eng = nc.gpsimd if et % 2 == 0 else nc.vector
                eng.tensor_add(kTm[et][:], kraw[et][:], mask_sb[:])
            return kTm

        def s_stats(rs, pad_sb):
            # row-sum -> subtract pad-column contribution -> reciprocal
            rsv = stpool.tile([P, 1], F32, tag="rsv")
            nc.vector.tensor_tensor(
                out=rsv[:], in0=rs[:], in1=pad_sb[:],
                op=mybir.AluOpType.subtract,
            )
            rc = stpool.tile([P, 1], F32, tag="recip")
            nc.vector.reciprocal(out=rc[:], in_=rsv[:])
            return rc

        def s_block(b, ib, qTt, kTm, pad_sb):
            sp = spsum.tile([P, SPAD], F32, tag="S")
            for (c0, cw) in kchunks:
                if s_fp8:
                    for j in range(NET // 2):
                        nc.tensor.matmul(
                            sp[:, c0:c0 + cw],
                            lhsT=qTt[j][:, 0:2, ib * P:(ib + 1) * P],
                            rhs=kTm[j][:, 0:2, c0:c0 + cw],
                            start=(j == 0),
                            stop=(j == NET // 2 - 1),
                            perf_mode=mybir.MatmulPerfMode.DoubleRow,
                        )
                else:
                    for et in range(NET):
                        nc.tensor.matmul(
                            sp[:, c0:c0 + cw],
                            lhsT=qTt[et][:, ib * P:(ib + 1) * P],
                            rhs=kTm[et][:, c0:c0 + cw],
                            start=(et == 0),
                            stop=(et == NET - 1),
                        )
            ex = epool.tile([P, NKP], BF16, tag="exp")
            rs = stpool.tile([P, 1], F32, tag="rowsum")
            nc.scalar.activation(
                out=ex[:], in_=sp[:, 0:NKP], func=AF.Exp, scale=SCALE,
                accum_out=rs[:],
            )
            rc = s_stats(rs, pad_sb)
            po = opool.tile([P, NKP], BF16, tag="po")
            nc.vector.tensor_scalar(
                out=po[:], in0=ex[:], scalar1=rc[:], scalar2=None,
                op0=mybir.AluOpType.mult,
            )
            # alternate store queues so the output backlog drains 2x faster
            # (sync, not scalar: scalar's ACT must not stall behind DMA issue)
            eng = nc.gpsimd if ib % 2 == 0 else nc.sync
            eng.dma_start(out=out[b, ib * P:(ib + 1) * P, :], in_=po[:])

        def s_block_final(b, ib, qTt, kTm, pad_sb):
            # Last block of the kernel: chunk-major matmuls into separate
            # 1-bank psums + a fully split epilogue so the first chunk's
            # exp/mul/store overlap the second chunk's matmuls and exp --
            # shortening the serial tail after the last MM.
            nch = len(kchunks)
            sps, rss, exs = [], [], []
            for ci, (c0, cw) in enumerate(kchunks):
                sps.append(ppsum.tile([P, 512], F32, tag="proj",
                                      name=f"fsp{ci}"))
                rss.append(stpool.tile([P, 1], F32, tag=f"rowsum{ci}",
                                       name=f"frs{ci}"))
                exs.append(epool.tile([P, cw], BF16, tag=f"fex{ci}",
                                      name=f"fex{ci}"))
            for ci, (c0, cw) in enumerate(kchunks):
                if s_fp8:
                    for j in range(NET // 2):
                        nc.tensor.matmul(
                            sps[ci][:, 0:cw],
                            lhsT=qTt[j][:, 0:2, ib * P:(ib + 1) * P],
                            rhs=kTm[j][:, 0:2, c0:c0 + cw],
                            start=(j == 0),
                            stop=(j == NET // 2 - 1),
                            perf_mode=mybir.MatmulPerfMode.DoubleRow,
                        )
                else:
                    for et in range(NET):
                        nc.tensor.matmul(
                            sps[ci][:, 0:cw],
                            lhsT=qTt[et][:, ib * P:(ib + 1) * P],
                            rhs=kTm[et][:, c0:c0 + cw],
                            start=(et == 0),
                            stop=(et == NET - 1),
                        )
                nc.scalar.activation(
                    out=exs[ci][:], in_=sps[ci][:, 0:cw],
                    func=AF.Exp, scale=SCALE, accum_out=rss[ci][:],
                )
            rs = rss[0]
            for ci in range(1, nch):
                rst = stpool.tile([P, 1], F32, tag="rowsumt", name=f"frt{ci}")
                nc.vector.tensor_tensor(
                    out=rst[:], in0=rs[:], in1=rss[ci][:],
                    op=mybir.AluOpType.add)
                rs = rst
            rc = s_stats(rs, pad_sb)
            for ci, (c0, cw) in enumerate(kchunks):
                poh = opool.tile([P, cw], BF16, tag=f"fpo{ci}", name=f"fpo{ci}")
                nc.vector.tensor_scalar(
                    out=poh[:], in0=exs[ci][:],
                    scalar1=rc[:], scalar2=None,
                    op0=mybir.AluOpType.mult,
                )
                eng = nc.gpsimd if ci % 2 == 0 else nc.sync
                eng.dma_start(
                    out=out[b, ib * P:(ib + 1) * P, c0:c0 + cw],
                    in_=poh[:],
                )

        def s_phase(b, qTt, kTm, pad_sb):
            for ib in range(NQB):
                if b == BL - 1 and ib == NQB - 1:
                    s_block_final(b, ib, qTt, kTm, pad_sb)
                else:
                    s_block(b, ib, qTt, kTm, pad_sb)

        cur = load_inputs(0)
        for b in range(BL):
            xk, xq, pad_sb, mask_sb = cur
            if use_mask:
                ktag = "kraw"
            else:
                ktag = "kTm"
            if s_fp8:
                kraw = [actpool.tile([P, 2, NKP], FP8, tag=f"{ktag}{j}",
                                     name=f"{ktag}{j}_{b}")
                        for j in range(NET // 2)]
            else:
                kraw = [actpool.tile([P, NKP], BF16, tag=f"{ktag}{et}",
                                     name=f"{ktag}{et}_{b}")
                        for et in range(NET)]
            if b == 0:
                proj_coldstart(xk, wk_sb, bk_sb, kraw, kchunks, pfx="coldk")
            else:
                proj(xk, wk_sb, bk_sb, kraw, kchunks)
            kTm = mask_add(kraw, mask_sb, b) if use_mask else kraw
            if s_fp8:
                qTt = [actpool.tile([P, 2, NQP], FP8, tag=f"qT{j}",
                                    name=f"qT{j}_{b}")
                       for j in range(NET // 2)]
            else:
                qTt = [actpool.tile([P, NQP], BF16, tag=f"qT{et}",
                                    name=f"qT{et}_{b}")
                       for et in range(NET)]
            if b == 0:
                proj_coldstart(xq, wq_sb, bq_sb, qTt, qchunks, pfx="coldq")
            else:
                proj(xq, wq_sb, bq_sb, qTt, qchunks)
            if b + 1 < BL:
                cur = load_inputs(b + 1)
            s_phase(b, qTt, kTm, pad_sb)


def _build(NQP, NKP, use_mask):
    nc = bacc.Bacc(
        "TRN2",
        target_bir_lowering=False,
        debug=False,
        enable_asserts=False,
        num_devices=NCORES,
    )
    qT = nc.dram_tensor("qT", [BL, D, NQP], BF16, kind="ExternalInput").ap()
    kT = nc.dram_tensor("kT", [BL, D, NKP], BF16, kind="ExternalInput").ap()
    Wq = nc.dram_tensor("Wq", [D, D], BF16, kind="ExternalInput").ap()
    Wk = nc.dram_tensor("Wk", [D, D], BF16, kind="ExternalInput").ap()
    bq = nc.dram_tensor("bq", [P, NET], F32, kind="ExternalInput").ap()
    bk = nc.dram_tensor("bk", [P, NET], F32, kind="ExternalInput").ap()
    padc = nc.dram_tensor("padc", [BL, P, 1], F32, kind="ExternalInput").ap()
    maskc = None
    if use_mask:
        maskc = nc.dram_tensor(
            "maskc", [BL, P, NKP], BF16, kind="ExternalInput").ap()
    out = nc.dram_tensor("out", [BL, NQP, NKP], BF16, kind="ExternalOutput").ap()

    with tile.TileContext(nc) as tc:
        _body(tc, qT, kT, Wq, Wk, bq, bk, padc, maskc, out, NQP, NKP)
    nc.compile()
    return nc


def _get_nc(NQP, NKP, use_mask):
    key = (NQP, NKP, use_mask)
    if key not in _CACHE:
        _CACHE[key] = _build(*key)
    return _CACHE[key]


def _pad128(n):
    return max(P, ((n + P - 1) // P) * P)


def _prep(query, key, query_mask, key_mask, Wq, bq, Wk, bk):
    bf = ml_dtypes.bfloat16
    query = np.asarray(query, dtype=np.float32)
    key = np.asarray(key, dtype=np.float32)
    qmask = np.asarray(query_mask) != 0
    kmask = np.asarray(key_mask) != 0
    qidx = [np.nonzero(qmask[g])[0] for g in range(B)]
    kidx = [np.nonzero(kmask[g])[0] for g in range(B)]
    NQP = _pad128(max(len(i) for i in qidx))
    NKP = _pad128(max(len(i) for i in kidx))
    use_mask = bool(np.any(np.asarray(bk, dtype=np.float32) != 0.0))

    Wq_b = np.asarray(Wq, dtype=np.float32).astype(bf)
    Wk_b = np.asarray(Wk, dtype=np.float32).astype(bf)
    # bias for feature e lives at partition e%128, column e//128
    bq_t = np.asarray(bq, dtype=np.float32).reshape(NET, P).T.copy()
    bk_t = np.asarray(bk, dtype=np.float32).reshape(NET, P).T.copy()

    in_maps = []
    for c in range(NCORES):
        qTc = np.zeros((BL, D, NQP), dtype=bf)
        kTc = np.zeros((BL, D, NKP), dtype=bf)
        padc = np.zeros((BL, P, 1), dtype=np.float32)
        imap = {"qT": qTc, "kT": kTc, "Wq": Wq_b, "Wk": Wk_b,
                "bq": bq_t, "bk": bk_t, "padc": padc}
        if use_mask:
            mk = np.zeros((BL, P, NKP), dtype=bf)
            imap["maskc"] = mk
        for b in range(BL):
            g = c * BL + b
            qi, ki = qidx[g], kidx[g]
            qTc[b, :, :len(qi)] = query[g][qi].T.astype(bf)
            kTc[b, :, :len(ki)] = key[g][ki].T.astype(bf)
            if use_mask:
                imap["maskc"][b, :, len(ki):] = bf(MASKC)
            else:
                padc[b, :, 0] = float(NKP - len(ki))
        in_maps.append(imap)
    return in_maps, qidx, kidx, NQP, NKP, use_mask


def run(query, key, query_mask, key_mask, Wq, bq, Wk, bk, **kwargs):
    """Run on hardware; returns (output, BassKernelResults)."""
    in_maps, qidx, kidx, NQP, NKP, use_mask = _prep(
        query, key, query_mask, key_mask, Wq, bq, Wk, bk)
    nc = _get_nc(NQP, NKP, use_mask)
    res = run_bass_kernel_spmd(nc, in_maps, core_ids=list(range(NCORES)),
                               **kwargs)
    full = np.zeros((B, LQ, LK), dtype=np.float32)
    for c in range(NCORES):
        oc = res.results[c]["out"]
        for b in range(BL):
            g = c * BL + b
            qi, ki = qidx[g], kidx[g]
            full[g][np.ix_(qi, ki)] = oc[b][:len(qi), :len(ki)].astype(np.float32)
    return full, res


def kernel(query, key, query_mask, key_mask, Wq, bq, Wk, bk):
    full, _ = run(query, key, query_mask, key_mask, Wq, bq, Wk, bk)
    return full
